# revision 4
# baseline (speedup 1.0000x reference)
"""Trainium2 Bass kernel for nn_LinearLoopLayer: out = x @ weight.T + bias.

x: (2048, 4096) f32, weight: (4096, 4096) f32, bias: (4096,) f32.
Sharding: 2 batch-halves x 4 out-feature-quarters across 8 NeuronCores.
Each core computes outT_shard[j, b] = sum_i wT[i, j] * xT[i, b] + bias[j]
as 512 bf16 matmuls (N=512, warm floor 109.2us @ 2.4GHz).

Trace-driven v2 (baseline 135.6us; measured decomposition: 10us startup +
3us HAM-cold + 106us gapless warm stream + 2.9us drain/store tail + 8.2us
framework sem-reset teardown):
  - The load phase is chip-HBM-bound (8 cores x 16.8MB with 4x replication
    of x / 2x of w = 134MB). The old xt-ring needed 150GB/s of a ~160GB/s
    contended fair share -> chunks landed marginally late -> one 3us stall
    that also re-throttled the PE clock (HAM) for another 3us.
  - v2 interleaves xt+wta chunks across BOTH HWDGE rings (sync+scalar) in
    exact consumption order (~112GB/s per ring, ~30% headroom), 1-ktile
    chunks early growing to 4-ktile late, wtb FIFO'd behind. No holdback
    memsets, no warmup flood: 12 warmup matmuls only (HAM flips to 2.4GHz
    ~3.4us after first PE activity; an early real-MM start at ~8.6us eats
    ~0.8us of cold instead of 3us).
  - Stores + bias ride the gpsimd SWDGE ring so the HWDGE rings stay
    pure-load; the final two stores use the by-then-idle sync ring (HWDGE
    has ~0.6us completion receipt vs ~2us SWDGE - the teardown barrier
    waits on the last store receipt).
  - Tail: last pass runs bank-sequential as bb0(512 cols) then bb1 as two
    256-col psum tiles; only a 256-col drain + 64KB store stays exposed.
"""

import sys

import numpy as np

sys.path.insert(0, "/opt/trn_rl_repo")

import concourse.mybir as mybir
from concourse import bacc, tile
from concourse.bass_utils import run_bass_kernel_spmd

P = 128
B, K, J = 2048, 4096, 4096
NCORES = 8
B_SPLIT, J_SPLIT = 2, 4
BL, JL = B // B_SPLIT, J // J_SPLIT  # per-core local batch / out-features
KT = K // P  # contraction tiles (32)
JB = JL // P  # 128-feature j-blocks per core (8)
NB = BL // 512  # 512-col batch blocks per core (2)
JH = JL // 2  # out-feature half (512) - wta/wtb split

# it-blocks for the interleaved load schedule: fine early (1 k-tile) so the
# first matmuls gate on tiny transfers, coarse late to amortize the ~0.6us
# per-trigger cost on the issuing queue. Blocks rotate across THREE rings
# (sync/scalar HWDGE + gpsimd SWDGE): the early phase is latency-bound at
# ~60-130GB/s per ring while all 8 cores' rings are in flight, so per-ring
# need must stay ~75GB/s. Within a ring the FIFO order equals consumption
# order.
IT_BLOCKS = [
    (1, 2), (2, 3), (3, 4),
    (4, 6), (6, 8), (8, 10), (10, 12),
    (12, 16), (16, 20), (20, 24), (24, 28), (28, 32),
]
# Warmup matmuls bridge PE activity from the preamble (~7.4us) until the
# first data-gated matmul (~11us): the HAM clock gate flips to 2.4GHz after
# ~3.4us of continuous activity, so the real stream starts warm and any
# overshoot costs only ~56ns per leftover warmup.
WARMUP_MMS = 36

PASS_A = (0, 1, 2, 3)  # 8 psum banks (x NB)
PASS_B = (4, 5, 6)     # 6 banks
# pass C (jb 7) is emitted bank-sequential with bb1 split into 2x256 cols

_NP_BF16 = mybir.dt.np(mybir.dt.bfloat16)


def _build():
    nc = bacc.Bacc(None, target_bir_lowering=False)
    bf16 = mybir.dt.bfloat16
    f32 = mybir.dt.float32
    xt = nc.declare_dram_parameter("xt", [P, KT * BL], bf16, isOutput=False)
    wta = nc.declare_dram_parameter("wta", [P, KT * JH], bf16, isOutput=False)
    wtb = nc.declare_dram_parameter("wtb", [P, KT * JH], bf16, isOutput=False)
    biasT = nc.declare_dram_parameter("biasT", [P, JB], f32, isOutput=False)
    # out stored as bf16 (host upcasts): halves store DMA and the exposed
    # final-store tail; adds ~1e-3 rel err on top of the input quantization
    out = nc.declare_dram_parameter("out", [JL, BL], bf16, isOutput=True)

    with tile.TileContext(nc) as tc:
        with (
            tc.tile_pool(name="xp", bufs=1) as xp,
            tc.tile_pool(name="wap", bufs=1) as wap,
            tc.tile_pool(name="wbp", bufs=1) as wbp,
            tc.tile_pool(name="biasp", bufs=1) as biasp,
            tc.tile_pool(name="outp", bufs=4) as outp,
            tc.tile_pool(name="psum", bufs=8, space="PSUM") as psum_pool,
        ):
            xt_sb = xp.tile([P, KT * BL], bf16)
            wta_sb = wap.tile([P, KT * JH], bf16)
            wtb_sb = wbp.tile([P, KT * JH], bf16)
            bias_sb = biasp.tile([P, JB], f32)

            # HAM warm-up: a few dummy matmuls start PE activity during the
            # preamble window so the clock gate flips to 2.4GHz ~3.4us after
            # first activity - right around when the first data-gated
            # matmuls begin.
            warm_sb = outp.tile([P, P], bf16, name="warm")
            nc.vector.memset(warm_sb[:], 0)
            warm_ps = psum_pool.tile([P, 512], f32, name="ps")
            for _ in range(WARMUP_MMS):
                nc.tensor.matmul(
                    warm_ps[:, :P], warm_sb[:], warm_sb[:],
                    start=True, stop=True,
                )

            # --- load schedule -------------------------------------------
            rings = [nc.gpsimd, nc.sync, nc.scalar]

            def load(ring, sb, dram, c0, c1):
                ring.dma_start(sb[:, c0:c1], dram[:, c0:c1])

            # k-tile 0 split so the very first matmul (jb0, bb0) gates on
            # only 32KB (wta cols 0:128) + 128KB (xt cols 0:512) on sync;
            # scalar carries the rest of k-tile 0.
            load(nc.sync, wta_sb, wta, 0, P)
            load(nc.sync, xt_sb, xt, 0, 512)
            load(nc.scalar, wta_sb, wta, P, JH)
            load(nc.scalar, xt_sb, xt, 512, 1024)
            for k, (a, b) in enumerate(IT_BLOCKS):
                ring = rings[k % 3]
                load(ring, wta_sb, wta, a * JH, b * JH)
                load(ring, xt_sb, xt, a * BL, b * BL)
            # wtb: FIFO behind the xt/wta stream (finishes ~45-52us, first
            # consumed ~63us), rotating rings; bias (tiny, consumed at the
            # first drain ~66us) last on gpsimd.
            for k in range(4):
                load(rings[k % 3], wtb_sb, wtb, k * 4096, (k + 1) * 4096)
            nc.gpsimd.dma_start(bias_sb[:], biasT[:, :])

            # --- compute -------------------------------------------------
            def drain_store(o_cols, ps_tile, jb, bcol0, drain_eng, st_eng):
                o = outp.tile([P, o_cols], bf16, name="o")
                if drain_eng is nc.vector:
                    nc.vector.tensor_scalar_add(
                        o[:], ps_tile[:], bias_sb[:, jb : jb + 1]
                    )
                else:
                    nc.scalar.activation(
                        o[:],
                        ps_tile[:],
                        mybir.ActivationFunctionType.Identity,
                        bias=bias_sb[:, jb : jb + 1],
                    )
                st_eng.dma_start(
                    out[jb * P : (jb + 1) * P, bcol0 : bcol0 + o_cols], o[:]
                )

            for pass_jbs in (PASS_A, PASS_B):
                ps = {
                    (jb, bb): psum_pool.tile([P, 512], f32, name="ps")
                    for jb in pass_jbs
                    for bb in range(NB)
                }
                for it in range(KT):
                    for jb in pass_jbs:
                        wsrc = wta_sb if jb < 4 else wtb_sb
                        jo = it * JH + (jb % 4) * P
                        for bb in range(NB):
                            nc.tensor.matmul(
                                ps[(jb, bb)][:],
                                wsrc[:, jo : jo + P],
                                xt_sb[:, it * BL + bb * 512 : it * BL + (bb + 1) * 512],
                                start=(it == 0),
                                stop=(it == KT - 1),
                            )
                for k, (jb, bb) in enumerate(
                    [(j, b) for j in pass_jbs for b in range(NB)]
                ):
                    drain_store(
                        512, ps[(jb, bb)], jb, bb * 512,
                        nc.vector if k % 2 == 0 else nc.scalar,
                        nc.gpsimd,
                    )

            # pass C (jb 7): bank-sequential so each unit's drain+store hide
            # under the next unit's compute; bb1 split into 2x256 cols so
            # the exposed tail is one 256-col drain + 64KB store.
            jb = 7
            ps_b0 = psum_pool.tile([P, 512], f32, name="ps")
            ps_b1a = psum_pool.tile([P, 256], f32, name="ps")
            ps_b1b = psum_pool.tile([P, 256], f32, name="ps")
            for it in range(KT):
                jo = it * JH + (jb % 4) * P
                nc.tensor.matmul(
                    ps_b0[:], wtb_sb[:, jo : jo + P],
                    xt_sb[:, it * BL : it * BL + 512],
                    start=(it == 0), stop=(it == KT - 1),
                )
            drain_store(512, ps_b0, jb, 0, nc.scalar, nc.gpsimd)
            for it in range(KT):
                jo = it * JH + (jb % 4) * P
                nc.tensor.matmul(
                    ps_b1a[:], wtb_sb[:, jo : jo + P],
                    xt_sb[:, it * BL + 512 : it * BL + 768],
                    start=(it == 0), stop=(it == KT - 1),
                )
            drain_store(256, ps_b1a, jb, 512, nc.vector, nc.sync)
            for it in range(KT):
                jo = it * JH + (jb % 4) * P
                nc.tensor.matmul(
                    ps_b1b[:], wtb_sb[:, jo : jo + P],
                    xt_sb[:, it * BL + 768 : it * BL + 1024],
                    start=(it == 0), stop=(it == KT - 1),
                )
            drain_store(256, ps_b1b, jb, 768, nc.vector, nc.sync)
    nc.finalize()
    return nc


_NC_CACHE = {}


def _get_nc():
    if "bf16" not in _NC_CACHE:
        _NC_CACHE["bf16"] = _build()
    return _NC_CACHE["bf16"]


def _part_major(a2d, cols):
    """[K, cols] f32 -> [P, KT*cols] bf16, i-tile-then-col per partition."""
    return np.ascontiguousarray(
        a2d.reshape(KT, P, cols).transpose(1, 0, 2).reshape(P, KT * cols)
    ).astype(_NP_BF16)


def _make_in_maps(x, weight, bias):
    x = np.asarray(x, dtype=np.float32)
    if x.ndim == 4:
        x = x.reshape(x.shape[0], -1)
    weight = np.asarray(weight, dtype=np.float32)
    bias = np.asarray(bias, dtype=np.float32)
    in_maps = []
    for c in range(NCORES):
        bh, jq = divmod(c, J_SPLIT)
        xT = x[bh * BL : (bh + 1) * BL].T  # [K, BL]
        wT = weight[jq * JL : (jq + 1) * JL].T  # [K, JL]
        bq = bias[jq * JL : (jq + 1) * JL]
        in_maps.append(
            {
                "xt": _part_major(xT, BL),
                "wta": _part_major(wT[:, :JH], JH),
                "wtb": _part_major(wT[:, JH:], JH),
                "biasT": np.ascontiguousarray(bq.reshape(JB, P).T),
            }
        )
    return in_maps


def _assemble(results):
    out = np.empty((B, J), dtype=np.float32)
    for c in range(NCORES):
        bh, jq = divmod(c, J_SPLIT)
        out[bh * BL : (bh + 1) * BL, jq * JL : (jq + 1) * JL] = (
            results[c]["out"].astype(np.float32).T
        )
    return out


def run(x, weight, bias, mm_dt_name=None, trace=False, **kwargs):
    nc = _get_nc()
    in_maps = _make_in_maps(x, weight, bias)
    res = run_bass_kernel_spmd(
        nc, in_maps, core_ids=list(range(NCORES)), trace=trace, **kwargs
    )
    return _assemble(res.results), res


def kernel(x, weight, bias):
    out, _ = run(x, weight, bias)
    return out


# revision 6
# speedup vs baseline: 1.1982x; 1.1982x over previous
"""Trainium2 Bass kernel for nn_LinearLoopLayer: out = x @ weight.T + bias.

x: (2048, 4096) f32, weight: (4096, 4096) f32, bias: (4096,) f32.
Sharding: 2 batch-halves x 4 out-feature-quarters across 8 NeuronCores.
Each core computes outT_shard[j, b] = sum_i wT[i, j] * xT[i, b] + bias[j]
as 512 bf16 matmuls (N=512, warm floor 109.2us @ 2.4GHz).

Trace-driven v2 (baseline 135.6us; measured decomposition: 10us startup +
3us HAM-cold + 106us gapless warm stream + 2.9us drain/store tail + 8.2us
framework sem-reset teardown):
  - The load phase is chip-HBM-bound (8 cores x 16.8MB with 4x replication
    of x / 2x of w = 134MB). The old xt-ring needed 150GB/s of a ~160GB/s
    contended fair share -> chunks landed marginally late -> one 3us stall
    that also re-throttled the PE clock (HAM) for another 3us.
  - v2 interleaves xt+wta chunks across BOTH HWDGE rings (sync+scalar) in
    exact consumption order (~112GB/s per ring, ~30% headroom), 1-ktile
    chunks early growing to 4-ktile late, wtb FIFO'd behind. No holdback
    memsets, no warmup flood: 12 warmup matmuls only (HAM flips to 2.4GHz
    ~3.4us after first PE activity; an early real-MM start at ~8.6us eats
    ~0.8us of cold instead of 3us).
  - Stores + bias ride the gpsimd SWDGE ring so the HWDGE rings stay
    pure-load; the final two stores use the by-then-idle sync ring (HWDGE
    has ~0.6us completion receipt vs ~2us SWDGE - the teardown barrier
    waits on the last store receipt).
  - Tail: last pass runs bank-sequential as bb0(512 cols) then bb1 as two
    256-col psum tiles; only a 256-col drain + 64KB store stays exposed.
"""

import sys

import numpy as np

sys.path.insert(0, "/opt/trn_rl_repo")

import concourse.mybir as mybir
from concourse import bacc, tile
from concourse.bass_utils import run_bass_kernel_spmd

P = 128
B, K, J = 2048, 4096, 4096
NCORES = 8
B_SPLIT, J_SPLIT = 2, 4
BL, JL = B // B_SPLIT, J // J_SPLIT  # per-core local batch / out-features
KT = K // P  # contraction tiles (32)
JB = JL // P  # 128-feature j-blocks per core (8)
NB = BL // 512  # 512-col batch blocks per core (2)
JH = JL // 2  # out-feature half (512) - wta/wtb split

# it-blocks for the interleaved load schedule: fine early (1 k-tile) so the
# first matmuls gate on tiny transfers, coarse late to amortize the ~0.6us
# per-trigger cost on the issuing queue. Blocks rotate across THREE rings
# (sync/scalar HWDGE + gpsimd SWDGE): the early phase is latency-bound at
# ~60-130GB/s per ring while all 8 cores' rings are in flight, so per-ring
# need must stay ~75GB/s. Within a ring the FIFO order equals consumption
# order.
IT_BLOCKS = [
    (1, 2), (2, 3), (3, 4),
    (4, 6), (6, 8), (8, 10), (10, 12), (12, 14), (14, 16),
    (16, 18), (18, 20), (20, 22), (22, 24), (24, 26), (26, 28),
    (28, 30), (30, 32),
]
# Warmup matmuls bridge PE activity from the preamble (~7.4us) until the
# first data-gated matmul (~11us): the HAM clock gate flips to 2.4GHz after
# ~3.4us of continuous activity, so the real stream starts warm and any
# overshoot costs only ~56ns per leftover warmup.
WARMUP_MMS = 36

PASS_A = (0, 1, 2, 3)  # 8 psum banks (x NB)
PASS_B = (4, 5, 6)     # 6 banks
# pass C (jb 7) is emitted bank-sequential with bb1 split into 2x256 cols

_NP_BF16 = mybir.dt.np(mybir.dt.bfloat16)


def _build():
    nc = bacc.Bacc(None, target_bir_lowering=False)
    bf16 = mybir.dt.bfloat16
    f32 = mybir.dt.float32
    xt = nc.declare_dram_parameter("xt", [P, KT * BL], bf16, isOutput=False)
    wta = nc.declare_dram_parameter("wta", [P, KT * JH], bf16, isOutput=False)
    wtb = nc.declare_dram_parameter("wtb", [P, KT * JH], bf16, isOutput=False)
    biasT = nc.declare_dram_parameter("biasT", [P, JB], f32, isOutput=False)
    # out stored as bf16 (host upcasts): halves store DMA and the exposed
    # final-store tail; adds ~1e-3 rel err on top of the input quantization
    out = nc.declare_dram_parameter("out", [JL, BL], bf16, isOutput=True)

    with tile.TileContext(nc) as tc:
        with (
            tc.tile_pool(name="xp", bufs=1) as xp,
            tc.tile_pool(name="wap", bufs=1) as wap,
            tc.tile_pool(name="wbp", bufs=1) as wbp,
            tc.tile_pool(name="biasp", bufs=1) as biasp,
            tc.tile_pool(name="outp", bufs=4) as outp,
            tc.tile_pool(name="psum", bufs=8, space="PSUM") as psum_pool,
        ):
            xt_sb = xp.tile([P, KT * BL], bf16)
            wta_sb = wap.tile([P, KT * JH], bf16)
            wtb_sb = wbp.tile([P, KT * JH], bf16)
            bias_sb = biasp.tile([P, JB], f32)

            # HAM warm-up: a few dummy matmuls start PE activity during the
            # preamble window so the clock gate flips to 2.4GHz ~3.4us after
            # first activity - right around when the first data-gated
            # matmuls begin.
            warm_sb = outp.tile([P, P], bf16, name="warm")
            nc.vector.memset(warm_sb[:], 0)
            warm_ps = psum_pool.tile([P, 512], f32, name="ps")
            for _ in range(WARMUP_MMS):
                nc.tensor.matmul(
                    warm_ps[:, :P], warm_sb[:], warm_sb[:],
                    start=True, stop=True,
                )

            # --- load schedule -------------------------------------------
            rings = [nc.gpsimd, nc.sync, nc.scalar]

            def load(ring, sb, dram, c0, c1):
                ring.dma_start(sb[:, c0:c1], dram[:, c0:c1])

            # k-tile 0 split so the very first matmul (jb0, bb0) gates on
            # only 32KB (wta cols 0:128) + 128KB (xt cols 0:512) on sync;
            # scalar carries the rest of k-tile 0.
            load(nc.sync, wta_sb, wta, 0, P)
            load(nc.sync, xt_sb, xt, 0, 512)
            load(nc.scalar, wta_sb, wta, P, JH)
            load(nc.scalar, xt_sb, xt, 512, 1024)
            for k, (a, b) in enumerate(IT_BLOCKS):
                ring = rings[k % 3]
                load(ring, wta_sb, wta, a * JH, b * JH)
                load(ring, xt_sb, xt, a * BL, b * BL)
            # wtb (4.2MB, first consumed ~63us) rides gpsimd exclusively so
            # the HWDGE rings stay pure just-in-time pass-A data; bias
            # (tiny, consumed at the first drain ~66us) follows it.
            for k in range(4):
                load(nc.gpsimd, wtb_sb, wtb, k * 4096, (k + 1) * 4096)
            nc.gpsimd.dma_start(bias_sb[:], biasT[:, :])

            # --- compute -------------------------------------------------
            def drain_store(o_cols, ps_tile, jb, bcol0, drain_eng, st_eng):
                o = outp.tile([P, o_cols], bf16, name="o")
                if drain_eng is nc.vector:
                    nc.vector.tensor_scalar_add(
                        o[:], ps_tile[:], bias_sb[:, jb : jb + 1]
                    )
                else:
                    nc.scalar.activation(
                        o[:],
                        ps_tile[:],
                        mybir.ActivationFunctionType.Identity,
                        bias=bias_sb[:, jb : jb + 1],
                    )
                st_eng.dma_start(
                    out[jb * P : (jb + 1) * P, bcol0 : bcol0 + o_cols], o[:]
                )

            for pass_jbs in (PASS_A, PASS_B):
                ps = {
                    (jb, bb): psum_pool.tile([P, 512], f32, name="ps")
                    for jb in pass_jbs
                    for bb in range(NB)
                }
                for it in range(KT):
                    for jb in pass_jbs:
                        wsrc = wta_sb if jb < 4 else wtb_sb
                        jo = it * JH + (jb % 4) * P
                        for bb in range(NB):
                            nc.tensor.matmul(
                                ps[(jb, bb)][:],
                                wsrc[:, jo : jo + P],
                                xt_sb[:, it * BL + bb * 512 : it * BL + (bb + 1) * 512],
                                start=(it == 0),
                                stop=(it == KT - 1),
                            )
                for k, (jb, bb) in enumerate(
                    [(j, b) for j in pass_jbs for b in range(NB)]
                ):
                    drain_store(
                        512, ps[(jb, bb)], jb, bb * 512,
                        nc.vector if k % 2 == 0 else nc.scalar,
                        nc.gpsimd,
                    )

            # pass C (jb 7): bank-sequential so each unit's drain+store hide
            # under the next unit's compute; bb1 split into 2x256 cols so
            # the exposed tail is one 256-col drain + 64KB store.
            jb = 7
            ps_b0 = psum_pool.tile([P, 512], f32, name="ps")
            ps_b1a = psum_pool.tile([P, 256], f32, name="ps")
            ps_b1b = psum_pool.tile([P, 256], f32, name="ps")
            for it in range(KT):
                jo = it * JH + (jb % 4) * P
                nc.tensor.matmul(
                    ps_b0[:], wtb_sb[:, jo : jo + P],
                    xt_sb[:, it * BL : it * BL + 512],
                    start=(it == 0), stop=(it == KT - 1),
                )
            drain_store(512, ps_b0, jb, 0, nc.scalar, nc.gpsimd)
            for it in range(KT):
                jo = it * JH + (jb % 4) * P
                nc.tensor.matmul(
                    ps_b1a[:], wtb_sb[:, jo : jo + P],
                    xt_sb[:, it * BL + 512 : it * BL + 768],
                    start=(it == 0), stop=(it == KT - 1),
                )
            drain_store(256, ps_b1a, jb, 512, nc.vector, nc.sync)
            for it in range(KT):
                jo = it * JH + (jb % 4) * P
                nc.tensor.matmul(
                    ps_b1b[:], wtb_sb[:, jo : jo + P],
                    xt_sb[:, it * BL + 768 : it * BL + 1024],
                    start=(it == 0), stop=(it == KT - 1),
                )
            drain_store(256, ps_b1b, jb, 768, nc.vector, nc.sync)
    nc.finalize()
    return nc


_NC_CACHE = {}


def _get_nc():
    if "bf16" not in _NC_CACHE:
        _NC_CACHE["bf16"] = _build()
    return _NC_CACHE["bf16"]


def _part_major(a2d, cols):
    """[K, cols] f32 -> [P, KT*cols] bf16, i-tile-then-col per partition."""
    return np.ascontiguousarray(
        a2d.reshape(KT, P, cols).transpose(1, 0, 2).reshape(P, KT * cols)
    ).astype(_NP_BF16)


def _make_in_maps(x, weight, bias):
    x = np.asarray(x, dtype=np.float32)
    if x.ndim == 4:
        x = x.reshape(x.shape[0], -1)
    weight = np.asarray(weight, dtype=np.float32)
    bias = np.asarray(bias, dtype=np.float32)
    in_maps = []
    for c in range(NCORES):
        bh, jq = divmod(c, J_SPLIT)
        xT = x[bh * BL : (bh + 1) * BL].T  # [K, BL]
        wT = weight[jq * JL : (jq + 1) * JL].T  # [K, JL]
        bq = bias[jq * JL : (jq + 1) * JL]
        in_maps.append(
            {
                "xt": _part_major(xT, BL),
                "wta": _part_major(wT[:, :JH], JH),
                "wtb": _part_major(wT[:, JH:], JH),
                "biasT": np.ascontiguousarray(bq.reshape(JB, P).T),
            }
        )
    return in_maps


def _assemble(results):
    out = np.empty((B, J), dtype=np.float32)
    for c in range(NCORES):
        bh, jq = divmod(c, J_SPLIT)
        out[bh * BL : (bh + 1) * BL, jq * JL : (jq + 1) * JL] = (
            results[c]["out"].astype(np.float32).T
        )
    return out


def run(x, weight, bias, mm_dt_name=None, trace=False, **kwargs):
    nc = _get_nc()
    in_maps = _make_in_maps(x, weight, bias)
    res = run_bass_kernel_spmd(
        nc, in_maps, core_ids=list(range(NCORES)), trace=trace, **kwargs
    )
    return _assemble(res.results), res


def kernel(x, weight, bias):
    out, _ = run(x, weight, bias)
    return out


# revision 8
# speedup vs baseline: 1.1986x; 1.0003x over previous
"""Trainium2 Bass kernel for nn_LinearLoopLayer: out = x @ weight.T + bias.

x: (2048, 4096) f32, weight: (4096, 4096) f32, bias: (4096,) f32.
Sharding: 2 batch-halves x 4 out-feature-quarters across 8 NeuronCores.
Each core computes outT_shard[j, b] = sum_i wT[i, j] * xT[i, b] + bias[j]
as 512 bf16 matmuls (N=512, warm floor 109.2us @ 2.4GHz).

Trace-driven v2 (baseline 135.6us; measured decomposition: 10us startup +
3us HAM-cold + 106us gapless warm stream + 2.9us drain/store tail + 8.2us
framework sem-reset teardown):
  - The load phase is chip-HBM-bound (8 cores x 16.8MB with 4x replication
    of x / 2x of w = 134MB). The old xt-ring needed 150GB/s of a ~160GB/s
    contended fair share -> chunks landed marginally late -> one 3us stall
    that also re-throttled the PE clock (HAM) for another 3us.
  - v2 interleaves xt+wta chunks across BOTH HWDGE rings (sync+scalar) in
    exact consumption order (~112GB/s per ring, ~30% headroom), 1-ktile
    chunks early growing to 4-ktile late, wtb FIFO'd behind. No holdback
    memsets, no warmup flood: 12 warmup matmuls only (HAM flips to 2.4GHz
    ~3.4us after first PE activity; an early real-MM start at ~8.6us eats
    ~0.8us of cold instead of 3us).
  - Stores + bias ride the gpsimd SWDGE ring so the HWDGE rings stay
    pure-load; the final two stores use the by-then-idle sync ring (HWDGE
    has ~0.6us completion receipt vs ~2us SWDGE - the teardown barrier
    waits on the last store receipt).
  - Tail: last pass runs bank-sequential as bb0(512 cols) then bb1 as two
    256-col psum tiles; only a 256-col drain + 64KB store stays exposed.
"""

import sys

import numpy as np

sys.path.insert(0, "/opt/trn_rl_repo")

import concourse.mybir as mybir
from concourse import bacc, tile
from concourse.bass_utils import run_bass_kernel_spmd

P = 128
B, K, J = 2048, 4096, 4096
NCORES = 8
B_SPLIT, J_SPLIT = 2, 4
BL, JL = B // B_SPLIT, J // J_SPLIT  # per-core local batch / out-features
KT = K // P  # contraction tiles (32)
JB = JL // P  # 128-feature j-blocks per core (8)
NB = BL // 512  # 512-col batch blocks per core (2)
JH = JL // 2  # out-feature half (512) - wta/wtb split

# it-blocks for the interleaved load schedule: fine early (1 k-tile) so the
# first matmuls gate on tiny transfers, coarse late to amortize the ~0.6us
# per-trigger cost on the issuing queue. Blocks rotate across THREE rings
# (sync/scalar HWDGE + gpsimd SWDGE): the early phase is latency-bound at
# ~60-130GB/s per ring while all 8 cores' rings are in flight, so per-ring
# need must stay ~75GB/s. Within a ring the FIFO order equals consumption
# order.
FINE_ITS = 6  # per-k-tile pieces for its 0..5, 2-it pairs afterwards
# Warmup matmuls bridge PE activity from the preamble (~7.4us) until the
# first data-gated matmul (~11us): the HAM clock gate flips to 2.4GHz after
# ~3.4us of continuous activity, so the real stream starts warm and any
# overshoot costs only ~56ns per leftover warmup.
WARMUP_MMS = 36

PASS_A = (0, 1, 2, 3)  # 8 psum banks (x NB)
PASS_B = (4, 5, 6)     # 6 banks
# pass C (jb 7) is emitted bank-sequential with bb1 split into 2x256 cols

_NP_BF16 = mybir.dt.np(mybir.dt.bfloat16)


def _build():
    nc = bacc.Bacc(None, target_bir_lowering=False)
    bf16 = mybir.dt.bfloat16
    f32 = mybir.dt.float32
    xt = nc.declare_dram_parameter("xt", [P, KT * BL], bf16, isOutput=False)
    wta = nc.declare_dram_parameter("wta", [P, KT * JH], bf16, isOutput=False)
    wtb = nc.declare_dram_parameter("wtb", [P, KT * JH], bf16, isOutput=False)
    biasT = nc.declare_dram_parameter("biasT", [P, JB], f32, isOutput=False)
    # out stored as bf16 (host upcasts): halves store DMA and the exposed
    # final-store tail; adds ~1e-3 rel err on top of the input quantization
    out = nc.declare_dram_parameter("out", [JL, BL], bf16, isOutput=True)

    with tile.TileContext(nc) as tc:
        with (
            tc.tile_pool(name="xp", bufs=1) as xp,
            tc.tile_pool(name="wap", bufs=1) as wap,
            tc.tile_pool(name="wbp", bufs=1) as wbp,
            tc.tile_pool(name="biasp", bufs=1) as biasp,
            tc.tile_pool(name="outp", bufs=4) as outp,
            tc.tile_pool(name="psum", bufs=8, space="PSUM") as psum_pool,
        ):
            xt_sb = xp.tile([P, KT * BL], bf16)
            wta_sb = wap.tile([P, KT * JH], bf16)
            wtb_sb = wbp.tile([P, KT * JH], bf16)
            bias_sb = biasp.tile([P, JB], f32)

            # HAM warm-up: a few dummy matmuls start PE activity during the
            # preamble window so the clock gate flips to 2.4GHz ~3.4us after
            # first activity - right around when the first data-gated
            # matmuls begin.
            warm_sb = outp.tile([P, P], bf16, name="warm")
            nc.vector.memset(warm_sb[:], 0)
            warm_ps = psum_pool.tile([P, 512], f32, name="ps")
            for _ in range(WARMUP_MMS):
                nc.tensor.matmul(
                    warm_ps[:, :P], warm_sb[:], warm_sb[:],
                    start=True, stop=True,
                )

            # --- load schedule -------------------------------------------
            # Every k-tile splits into three equal 128KB pieces (wta_k /
            # xt_k first half / xt_k second half), one per ring with a
            # rotating assignment: each ring carries exactly 128KB per
            # k-tile (75GB/s uniform need vs the ~70-130GB/s per-ring rate
            # observed while all 8 cores' rings are in flight), FIFO'd in
            # deadline order. k-tiles 6+ use 2-tile pairs (256KB pieces) to
            # amortize the ~0.6-0.8us per-trigger cost.
            rings = [nc.sync, nc.scalar, nc.gpsimd]

            def load(ring, sb, dram, c0, c1):
                ring.dma_start(sb[:, c0:c1], dram[:, c0:c1])

            # k-tile 0: the very first matmul (jb0, bb0) gates on wta cols
            # 0:128 (32KB) + xt cols 0:512 on sync.
            load(nc.sync, wta_sb, wta, 0, P)
            load(nc.sync, xt_sb, xt, 0, 512)
            load(nc.scalar, wta_sb, wta, P, JH)
            load(nc.gpsimd, xt_sb, xt, 512, 1024)
            for it in range(1, FINE_ITS):
                load(rings[it % 3], wta_sb, wta, it * JH, (it + 1) * JH)
                load(rings[(it + 1) % 3], xt_sb, xt, it * BL, it * BL + 512)
                load(rings[(it + 2) % 3], xt_sb, xt, it * BL + 512, (it + 1) * BL)
            for p, it in enumerate(range(FINE_ITS, KT, 2)):
                load(rings[p % 3], wta_sb, wta, it * JH, (it + 2) * JH)
                load(rings[(p + 1) % 3], xt_sb, xt, it * BL, (it + 1) * BL)
                load(rings[(p + 2) % 3], xt_sb, xt, (it + 1) * BL, (it + 2) * BL)
            # wtb (4.2MB, first consumed ~63us) FIFO-trails the pass-A
            # stream; bias (tiny, consumed at the first drain ~66us) last.
            for k in range(4):
                load(rings[k % 3], wtb_sb, wtb, k * 4096, (k + 1) * 4096)
            nc.gpsimd.dma_start(bias_sb[:], biasT[:, :])

            # --- compute -------------------------------------------------
            def drain_store(o_cols, ps_tile, jb, bcol0, drain_eng, st_eng):
                o = outp.tile([P, o_cols], bf16, name="o")
                if drain_eng is nc.vector:
                    nc.vector.tensor_scalar_add(
                        o[:], ps_tile[:], bias_sb[:, jb : jb + 1]
                    )
                else:
                    nc.scalar.activation(
                        o[:],
                        ps_tile[:],
                        mybir.ActivationFunctionType.Identity,
                        bias=bias_sb[:, jb : jb + 1],
                    )
                st_eng.dma_start(
                    out[jb * P : (jb + 1) * P, bcol0 : bcol0 + o_cols], o[:]
                )

            for pass_jbs in (PASS_A, PASS_B):
                ps = {
                    (jb, bb): psum_pool.tile([P, 512], f32, name="ps")
                    for jb in pass_jbs
                    for bb in range(NB)
                }
                for it in range(KT):
                    for jb in pass_jbs:
                        wsrc = wta_sb if jb < 4 else wtb_sb
                        jo = it * JH + (jb % 4) * P
                        for bb in range(NB):
                            nc.tensor.matmul(
                                ps[(jb, bb)][:],
                                wsrc[:, jo : jo + P],
                                xt_sb[:, it * BL + bb * 512 : it * BL + (bb + 1) * 512],
                                start=(it == 0),
                                stop=(it == KT - 1),
                            )
                for k, (jb, bb) in enumerate(
                    [(j, b) for j in pass_jbs for b in range(NB)]
                ):
                    drain_store(
                        512, ps[(jb, bb)], jb, bb * 512,
                        nc.vector if k % 2 == 0 else nc.scalar,
                        nc.gpsimd,
                    )

            # pass C (jb 7): bank-sequential so each unit's drain+store hide
            # under the next unit's compute; bb1 split into 2x256 cols so
            # the exposed tail is one 256-col drain + 64KB store.
            jb = 7
            ps_b0 = psum_pool.tile([P, 512], f32, name="ps")
            ps_b1a = psum_pool.tile([P, 256], f32, name="ps")
            ps_b1b = psum_pool.tile([P, 256], f32, name="ps")
            for it in range(KT):
                jo = it * JH + (jb % 4) * P
                nc.tensor.matmul(
                    ps_b0[:], wtb_sb[:, jo : jo + P],
                    xt_sb[:, it * BL : it * BL + 512],
                    start=(it == 0), stop=(it == KT - 1),
                )
            drain_store(512, ps_b0, jb, 0, nc.scalar, nc.gpsimd)
            for it in range(KT):
                jo = it * JH + (jb % 4) * P
                nc.tensor.matmul(
                    ps_b1a[:], wtb_sb[:, jo : jo + P],
                    xt_sb[:, it * BL + 512 : it * BL + 768],
                    start=(it == 0), stop=(it == KT - 1),
                )
            drain_store(256, ps_b1a, jb, 512, nc.vector, nc.sync)
            for it in range(KT):
                jo = it * JH + (jb % 4) * P
                nc.tensor.matmul(
                    ps_b1b[:], wtb_sb[:, jo : jo + P],
                    xt_sb[:, it * BL + 768 : it * BL + 1024],
                    start=(it == 0), stop=(it == KT - 1),
                )
            drain_store(256, ps_b1b, jb, 768, nc.vector, nc.sync)
    nc.finalize()
    return nc


_NC_CACHE = {}


def _get_nc():
    if "bf16" not in _NC_CACHE:
        _NC_CACHE["bf16"] = _build()
    return _NC_CACHE["bf16"]


def _part_major(a2d, cols):
    """[K, cols] f32 -> [P, KT*cols] bf16, i-tile-then-col per partition."""
    return np.ascontiguousarray(
        a2d.reshape(KT, P, cols).transpose(1, 0, 2).reshape(P, KT * cols)
    ).astype(_NP_BF16)


def _make_in_maps(x, weight, bias):
    x = np.asarray(x, dtype=np.float32)
    if x.ndim == 4:
        x = x.reshape(x.shape[0], -1)
    weight = np.asarray(weight, dtype=np.float32)
    bias = np.asarray(bias, dtype=np.float32)
    in_maps = []
    for c in range(NCORES):
        bh, jq = divmod(c, J_SPLIT)
        xT = x[bh * BL : (bh + 1) * BL].T  # [K, BL]
        wT = weight[jq * JL : (jq + 1) * JL].T  # [K, JL]
        bq = bias[jq * JL : (jq + 1) * JL]
        in_maps.append(
            {
                "xt": _part_major(xT, BL),
                "wta": _part_major(wT[:, :JH], JH),
                "wtb": _part_major(wT[:, JH:], JH),
                "biasT": np.ascontiguousarray(bq.reshape(JB, P).T),
            }
        )
    return in_maps


def _assemble(results):
    out = np.empty((B, J), dtype=np.float32)
    for c in range(NCORES):
        bh, jq = divmod(c, J_SPLIT)
        out[bh * BL : (bh + 1) * BL, jq * JL : (jq + 1) * JL] = (
            results[c]["out"].astype(np.float32).T
        )
    return out


def run(x, weight, bias, mm_dt_name=None, trace=False, **kwargs):
    nc = _get_nc()
    in_maps = _make_in_maps(x, weight, bias)
    res = run_bass_kernel_spmd(
        nc, in_maps, core_ids=list(range(NCORES)), trace=trace, **kwargs
    )
    return _assemble(res.results), res


def kernel(x, weight, bias):
    out, _ = run(x, weight, bias)
    return out


# revision 14
# speedup vs baseline: 1.2380x; 1.0329x over previous
"""Trainium2 Bass kernel for nn_LinearLoopLayer: out = x @ weight.T + bias.

x: (2048, 4096) f32, weight: (4096, 4096) f32, bias: (4096,) f32.
Sharding: 2 batch-halves x 4 out-feature-quarters across 8 NeuronCores.
Each core computes outT_shard[j, b] = sum_i wT[i, j] * xT[i, b] + bias[j]
as 512 bf16 matmuls (N=512, warm floor 109.2us @ 2.4GHz).

Trace-driven v2 (baseline 135.6us; measured decomposition: 10us startup +
3us HAM-cold + 106us gapless warm stream + 2.9us drain/store tail + 8.2us
framework sem-reset teardown):
  - The load phase is chip-HBM-bound (8 cores x 16.8MB with 4x replication
    of x / 2x of w = 134MB). The old xt-ring needed 150GB/s of a ~160GB/s
    contended fair share -> chunks landed marginally late -> one 3us stall
    that also re-throttled the PE clock (HAM) for another 3us.
  - v2 interleaves xt+wta chunks across BOTH HWDGE rings (sync+scalar) in
    exact consumption order (~112GB/s per ring, ~30% headroom), 1-ktile
    chunks early growing to 4-ktile late, wtb FIFO'd behind. No holdback
    memsets, no warmup flood: 12 warmup matmuls only (HAM flips to 2.4GHz
    ~3.4us after first PE activity; an early real-MM start at ~8.6us eats
    ~0.8us of cold instead of 3us).
  - Stores + bias ride the gpsimd SWDGE ring so the HWDGE rings stay
    pure-load; the final two stores use the by-then-idle sync ring (HWDGE
    has ~0.6us completion receipt vs ~2us SWDGE - the teardown barrier
    waits on the last store receipt).
  - Tail: last pass runs bank-sequential as bb0(512 cols) then bb1 as two
    256-col psum tiles; only a 256-col drain + 64KB store stays exposed.
"""

import sys

import numpy as np

sys.path.insert(0, "/opt/trn_rl_repo")

import concourse.mybir as mybir
from concourse import bacc, tile
from concourse.bass_utils import run_bass_kernel_spmd

P = 128
B, K, J = 2048, 4096, 4096
NCORES = 8
B_SPLIT, J_SPLIT = 2, 4
BL, JL = B // B_SPLIT, J // J_SPLIT  # per-core local batch / out-features
KT = K // P  # contraction tiles (32)
JB = JL // P  # 128-feature j-blocks per core (8)
NB = BL // 512  # 512-col batch blocks per core (2)
JH = JL // 2  # out-feature half (512) - wta/wtb split

# it-blocks for the interleaved load schedule: fine early (1 k-tile) so the
# first matmuls gate on tiny transfers, coarse late to amortize the ~0.6us
# per-trigger cost on the issuing queue. Blocks rotate across THREE rings
# (sync/scalar HWDGE + gpsimd SWDGE): the early phase is latency-bound at
# ~60-130GB/s per ring while all 8 cores' rings are in flight, so per-ring
# need must stay ~75GB/s. Within a ring the FIFO order equals consumption
# order.
WARMUP_MMS = 33  # see load schedule comment below


PASS_A = (0, 1, 2, 3)  # 8 psum banks (x NB)
PASS_B = (4, 5, 6)     # 6 banks
# pass C (jb 7) is emitted bank-sequential with bb1 split into 2x256 cols

_NP_BF16 = mybir.dt.np(mybir.dt.bfloat16)


def _build():
    nc = bacc.Bacc(None, target_bir_lowering=False)
    bf16 = mybir.dt.bfloat16
    f32 = mybir.dt.float32
    # tile-major DRAM layout: one [P, cols] tile per k-tile, so each load
    # piece reads adjacent-partition contiguous runs instead of 64KB-strided
    # 1-2KB segments (DRAM page locality for the latency-bound early phase)
    xt = nc.declare_dram_parameter("xt", [KT, P, BL], bf16, isOutput=False)
    wta = nc.declare_dram_parameter("wta", [KT, P, JH], bf16, isOutput=False)
    wtb = nc.declare_dram_parameter("wtb", [KT, P, JH], bf16, isOutput=False)
    biasT = nc.declare_dram_parameter("biasT", [P, JB], f32, isOutput=False)
    # out stored as bf16 (host upcasts): halves store DMA and the exposed
    # final-store tail; adds ~1e-3 rel err on top of the input quantization
    out = nc.declare_dram_parameter("out", [JL, BL], bf16, isOutput=True)

    with tile.TileContext(nc) as tc:
        with (
            tc.tile_pool(name="xp", bufs=1) as xp,
            tc.tile_pool(name="wap", bufs=1) as wap,
            tc.tile_pool(name="wbp", bufs=1) as wbp,
            tc.tile_pool(name="biasp", bufs=1) as biasp,
            tc.tile_pool(name="outp", bufs=4) as outp,
            tc.tile_pool(name="psum", bufs=8, space="PSUM") as psum_pool,
        ):
            xt_sb = xp.tile([P, KT * BL], bf16)
            wta_sb = wap.tile([P, KT * JH], bf16)
            wtb_sb = wbp.tile([P, KT * JH], bf16)
            bias_sb = biasp.tile([P, JB], f32)

            # HAM warm-up: a few dummy matmuls start PE activity during the
            # preamble window so the clock gate flips to 2.4GHz ~3.4us after
            # first activity - right around when the first data-gated
            # matmuls begin.
            warm_sb = outp.tile([P, P], bf16, name="warm")
            nc.vector.memset(warm_sb[:], 0)
            warm_ps = psum_pool.tile([P, 512], f32, name="ps")
            for _ in range(WARMUP_MMS):
                nc.tensor.matmul(
                    warm_ps[:, :P], warm_sb[:], warm_sb[:],
                    start=True, stop=True,
                )

            # --- load schedule -------------------------------------------
            # Per-k-tile pieces (xt_k 256KB / wta_k 128KB) rotate across the
            # three rings in deadline order: each ring carries 384KB per
            # 3-k-tile cycle (75GB/s uniform need vs the ~70-130GB/s
            # per-ring rate observed while all 8 cores' rings are in
            # flight). The warmup matmuls bridge PE activity from ~7.5us to
            # ~11us so the HAM clock gate (needs ~3.4us of gap-free PE
            # activity) flips to 2.4GHz right as the data-gated stream
            # starts.
            rings = [nc.sync, nc.scalar, nc.gpsimd]

            def load(ring, sb, dram, it, c0, c1, cols):
                ring.dma_start(
                    sb[:, it * cols + c0 : it * cols + c1],
                    dram[it, :, c0:c1],
                )

            # k-tile 0: the very first matmul (jb0, bb0) gates on wta cols
            # 0:128 (32KB) + xt cols 0:512 on sync.
            load(nc.sync, wta_sb, wta, 0, 0, P, JH)
            load(nc.sync, xt_sb, xt, 0, 0, 512, BL)
            load(nc.scalar, wta_sb, wta, 0, P, JH, JH)
            load(nc.gpsimd, xt_sb, xt, 0, 512, BL, BL)
            for it in range(1, KT):
                load(rings[it % 3], xt_sb, xt, it, 0, BL, BL)
                load(rings[(it + 1) % 3], wta_sb, wta, it, 0, JH, JH)
            # wtb (4.2MB, first consumed ~63us) FIFO-trails the pass-A
            # stream; bias (tiny, consumed at the first drain ~66us) last.
            for it in range(KT):
                load(rings[(it + 2) % 3], wtb_sb, wtb, it, 0, JH, JH)
            nc.gpsimd.dma_start(bias_sb[:], biasT[:, :])

            # --- compute -------------------------------------------------
            def drain_store(o_cols, ps_tile, jb, bcol0, drain_eng, st_eng):
                o = outp.tile([P, o_cols], bf16, name="o")
                if drain_eng is nc.vector:
                    nc.vector.tensor_scalar_add(
                        o[:], ps_tile[:], bias_sb[:, jb : jb + 1]
                    )
                else:
                    nc.scalar.activation(
                        o[:],
                        ps_tile[:],
                        mybir.ActivationFunctionType.Identity,
                        bias=bias_sb[:, jb : jb + 1],
                    )
                st_eng.dma_start(
                    out[jb * P : (jb + 1) * P, bcol0 : bcol0 + o_cols], o[:]
                )

            for pass_jbs in (PASS_A, PASS_B):
                ps = {
                    (jb, bb): psum_pool.tile([P, 512], f32, name="ps")
                    for jb in pass_jbs
                    for bb in range(NB)
                }
                for it in range(KT):
                    for jb in pass_jbs:
                        wsrc = wta_sb if jb < 4 else wtb_sb
                        jo = it * JH + (jb % 4) * P
                        for bb in range(NB):
                            nc.tensor.matmul(
                                ps[(jb, bb)][:],
                                wsrc[:, jo : jo + P],
                                xt_sb[:, it * BL + bb * 512 : it * BL + (bb + 1) * 512],
                                start=(it == 0),
                                stop=(it == KT - 1),
                            )
                for k, (jb, bb) in enumerate(
                    [(j, b) for j in pass_jbs for b in range(NB)]
                ):
                    drain_store(
                        512, ps[(jb, bb)], jb, bb * 512,
                        nc.vector if k % 2 == 0 else nc.scalar,
                        nc.gpsimd,
                    )

            # pass C (jb 7): bank-sequential so each unit's drain+store hide
            # under the next unit's compute; bb1 split into 2x256 cols so
            # the exposed tail is one 256-col drain + 64KB store.
            jb = 7
            ps_b0 = psum_pool.tile([P, 512], f32, name="ps")
            ps_b1a = psum_pool.tile([P, 256], f32, name="ps")
            ps_b1b = psum_pool.tile([P, 256], f32, name="ps")
            for it in range(KT):
                jo = it * JH + (jb % 4) * P
                nc.tensor.matmul(
                    ps_b0[:], wtb_sb[:, jo : jo + P],
                    xt_sb[:, it * BL : it * BL + 512],
                    start=(it == 0), stop=(it == KT - 1),
                )
            drain_store(512, ps_b0, jb, 0, nc.scalar, nc.gpsimd)
            for it in range(KT):
                jo = it * JH + (jb % 4) * P
                nc.tensor.matmul(
                    ps_b1a[:], wtb_sb[:, jo : jo + P],
                    xt_sb[:, it * BL + 512 : it * BL + 768],
                    start=(it == 0), stop=(it == KT - 1),
                )
            drain_store(256, ps_b1a, jb, 512, nc.vector, nc.sync)
            for it in range(KT):
                jo = it * JH + (jb % 4) * P
                nc.tensor.matmul(
                    ps_b1b[:], wtb_sb[:, jo : jo + P],
                    xt_sb[:, it * BL + 768 : it * BL + 1024],
                    start=(it == 0), stop=(it == KT - 1),
                )
            drain_store(256, ps_b1b, jb, 768, nc.vector, nc.sync)
    nc.finalize()
    return nc


_NC_CACHE = {}


def _get_nc():
    if "bf16" not in _NC_CACHE:
        _NC_CACHE["bf16"] = _build()
    return _NC_CACHE["bf16"]


def _part_major(a2d, cols):
    """[K, cols] f32 -> [KT, P, cols] bf16 (tile-major, contiguous tiles)."""
    return np.ascontiguousarray(a2d.reshape(KT, P, cols)).astype(_NP_BF16)


def _make_in_maps(x, weight, bias):
    x = np.asarray(x, dtype=np.float32)
    if x.ndim == 4:
        x = x.reshape(x.shape[0], -1)
    weight = np.asarray(weight, dtype=np.float32)
    bias = np.asarray(bias, dtype=np.float32)
    in_maps = []
    for c in range(NCORES):
        bh, jq = divmod(c, J_SPLIT)
        xT = x[bh * BL : (bh + 1) * BL].T  # [K, BL]
        wT = weight[jq * JL : (jq + 1) * JL].T  # [K, JL]
        bq = bias[jq * JL : (jq + 1) * JL]
        in_maps.append(
            {
                "xt": _part_major(xT, BL),
                "wta": _part_major(wT[:, :JH], JH),
                "wtb": _part_major(wT[:, JH:], JH),
                "biasT": np.ascontiguousarray(bq.reshape(JB, P).T),
            }
        )
    return in_maps


def _assemble(results):
    out = np.empty((B, J), dtype=np.float32)
    for c in range(NCORES):
        bh, jq = divmod(c, J_SPLIT)
        out[bh * BL : (bh + 1) * BL, jq * JL : (jq + 1) * JL] = (
            results[c]["out"].astype(np.float32).T
        )
    return out


def run(x, weight, bias, mm_dt_name=None, trace=False, **kwargs):
    nc = _get_nc()
    in_maps = _make_in_maps(x, weight, bias)
    res = run_bass_kernel_spmd(
        nc, in_maps, core_ids=list(range(NCORES)), trace=trace, **kwargs
    )
    return _assemble(res.results), res


def kernel(x, weight, bias):
    out, _ = run(x, weight, bias)
    return out
